# revision 9
# baseline (speedup 1.0000x reference)
"""CTC alignment distillation loss on 8 Trainium2 NeuronCores.

v4 strategy (transposed fp8 + PE reductions):
  * Only non-blank frames contribute (~2.4k of 8192 B*T positions).  All
    index math is host-side; frames are balanced round-robin across the
    8 cores (F = padded frames/core).
  * Device layout is TRANSPOSED: vocab on partitions, frames on the free
    axis.  The host packs, per core, two fp8 tensors [128, 63*F]:
      r8: logits[v, f]      (pad rows/cols hold -240 so exp() -> 0)
      s8: SCALE*(W*soft[lm_f, v] + (1-W)*A_R + C_Y*delta(v == y_f))
    The delta fold makes sum_v s8*r == SCALE*(contrib_f + lse_f), i.e.
    the label-smoothing algebra AND the ry gather ride along for free.
  * Per pass the device computes, chunked over 3 groups of 21 v-chunks:
      DVE : prod = r8 * s8            (fp8 in, bf16 out, 1x rate)
      ACT : e    = exp(r8)            (bf16 out; no table switches)
      PE  : ones-matmuls accumulate psum_dot[1,F]  += col-sums of prod
                                      psum_esum[1,F] += col-sums of e
    i.e. both V-reductions run on the otherwise-idle TensorEngine.
  * Host: contrib_f = dot_f/SCALE - log(esum_f); loss = -sum w_f*contrib.
"""

import numpy as np
from contextlib import ExitStack

import ml_dtypes

B, T, V = 16, 512, 8000
BLANK = 0
LSM = 0.1
W_SOFT = 0.5
N_CORES = 8
P = 128
NCHUNK = 63              # ceil(V/128)
VP = NCHUNK * P          # 8064
NGROUP = 3               # DMA/compute groups per pass
CG = NCHUNK // NGROUP    # 21 v-chunks per group
SCALE = 256.0            # slab scale so fp8(e4m3) covers the delta spike
PAD_LOGIT = -240.0       # most-negative TRN e4m3 normal; exp() == 0

A_R = LSM / (V - 1)
A_Y = (1.0 - LSM) - A_R
C_Y = (1.0 - W_SOFT) * A_Y

FP8_NP = ml_dtypes.float8_e4m3   # TRN float8e4-compatible (inf/240 max)

_PROGRAM_CACHE: dict = {}


def _build_program(F, reps: int = 1, variant: str = "v4", loop_reps: int = 0):
    """Bass/Tile program; F = frames per core (multiple of 8).

    variant: "v4"      - full kernel
             "v4dma"   - DMA only (roofline probe)
             "v4nodma" - compute only (resident tiles)
             "v4nope"  - DMA + DVE + ACT, no PE matmuls
    """
    import concourse.bass as bass
    import concourse.tile as tile
    from concourse import bacc, mybir

    f32 = mybir.dt.float32
    bf16 = mybir.dt.bfloat16
    fp8 = mybir.dt.float8e4
    ALU = mybir.AluOpType
    ACTF = mybir.ActivationFunctionType

    do_dma = variant != "v4nodma"
    do_compute = variant != "v4dma"
    do_pe = variant not in ("v4dma", "v4nope")

    GFD = CG * F            # free size of one group tile
    GS = 13                 # chunks per group multiplied on DVE; rest on GPSIMD
    DFD = GS * F
    QFD = (CG - GS) * F

    nc = bacc.Bacc(
        "TRN2", target_bir_lowering=False, debug=False, num_devices=N_CORES
    )
    r_d = nc.dram_tensor("r8", [P, NCHUNK * F], fp8, kind="ExternalInput")
    s_d = nc.dram_tensor("s8", [P, NCHUNK * F], fp8, kind="ExternalInput")
    stats_d = nc.dram_tensor("stats", [1, 2 * F], f32, kind="ExternalOutput")

    with tile.TileContext(nc) as tc, ExitStack() as ctx:
        rpool = ctx.enter_context(tc.tile_pool(name="R", bufs=3))
        spool = ctx.enter_context(tc.tile_pool(name="S", bufs=3))
        ppool = ctx.enter_context(tc.tile_pool(name="PR", bufs=2))
        qpool = ctx.enter_context(tc.tile_pool(name="Q", bufs=2))
        epool = ctx.enter_context(tc.tile_pool(name="E", bufs=2))
        const = ctx.enter_context(tc.tile_pool(name="C", bufs=1))
        opool = ctx.enter_context(tc.tile_pool(name="O", bufs=2))
        pspool = ctx.enter_context(tc.psum_pool(name="PS", bufs=2))

        ones = const.tile([P, 1], bf16)
        nc.gpsimd.memset(ones[:], 1.0)

        resident = []
        if not do_dma:
            for g in range(NGROUP):
                Rt = rpool.tile([P, GFD], fp8, name=f"Rres{g}")
                St = spool.tile([P, GFD], fp8, name=f"Sres{g}")
                nc.gpsimd.memset(Rt[:], -1.0)
                nc.gpsimd.memset(St[:], 0.25)
                resident.append((Rt, St))

        def emit_pass():
            ps_dot = pspool.tile([1, F], f32, name="ps_dot")
            ps_esum = pspool.tile([1, F], f32, name="ps_esum")
            stats_sb = opool.tile([1, 2 * F], f32, name="stats_sb")
            for g in range(NGROUP):
                o = g * GFD
                if do_dma:
                    Rt = rpool.tile([P, GFD], fp8, name="Rt")
                    St = spool.tile([P, GFD], fp8, name="St")
                    nc.sync.dma_start(Rt[:], r_d.ap()[:, o:o + GFD])
                    nc.scalar.dma_start(St[:], s_d.ap()[:, o:o + GFD])
                else:
                    Rt, St = resident[g]
                if not do_compute:
                    continue
                Pt = ppool.tile([P, DFD], bf16, name="Pt")
                Qt = qpool.tile([P, QFD], bf16, name="Qt")
                Et = epool.tile([P, GFD], bf16, name="Et")
                nc.vector.tensor_tensor(
                    out=Pt[:], in0=Rt[:, :DFD], in1=St[:, :DFD], op=ALU.mult)
                nc.gpsimd.tensor_tensor(
                    out=Qt[:], in0=Rt[:, DFD:], in1=St[:, DFD:], op=ALU.mult)
                nc.scalar.activation(out=Et[:], in_=Rt[:], func=ACTF.Exp)
                if do_pe:
                    for cc in range(CG):
                        c = g * CG + cc
                        prod_rhs = (Pt[:, cc * F:(cc + 1) * F] if cc < GS
                                    else Qt[:, (cc - GS) * F:(cc - GS + 1) * F])
                        nc.tensor.matmul(
                            out=ps_dot[:], lhsT=ones[:],
                            rhs=prod_rhs,
                            start=(c == 0), stop=(c == NCHUNK - 1),
                        )
                        nc.tensor.matmul(
                            out=ps_esum[:], lhsT=ones[:],
                            rhs=Et[:, cc * F:(cc + 1) * F],
                            start=(c == 0), stop=(c == NCHUNK - 1),
                        )
            if do_pe:
                nc.vector.tensor_copy(stats_sb[0:1, 0:F], ps_dot[:])
                nc.vector.tensor_copy(stats_sb[0:1, F:2 * F], ps_esum[:])
            else:
                nc.gpsimd.memset(stats_sb[:], 1.0)
            nc.sync.dma_start(stats_d.ap(), stats_sb[:])

        if loop_reps:
            with tc.For_i(0, loop_reps, 1):
                for _ in range(reps):
                    emit_pass()
        else:
            for _ in range(reps):
                emit_pass()

    nc.compile()
    return nc


def _host_prep(ys, aligns, xlens):
    """Reference index math -> flat frame list, balanced across cores."""
    frame_mask = np.arange(T)[None, :] < xlens[:, None]
    a = np.where(frame_mask, aligns, BLANK)
    nonblank = a != BLANK
    shifted = np.concatenate([np.full((B, 1), BLANK, a.dtype), a[:, :-1]], axis=1)
    run_start = nonblank & (a != shifted)
    label_id = np.cumsum(run_start.astype(np.int64), axis=1) - 1
    lm = np.maximum(label_id, 0)
    n_exists = nonblank.sum(axis=1)

    b_idx, t_idx = np.nonzero(nonblank)
    lm_f = lm[b_idx, t_idx]
    y_f = ys[b_idx, lm_f]
    w_f = 1.0 / (B * n_exists[b_idx].astype(np.float64))

    cores = []
    for c in range(N_CORES):
        sel = slice(c, None, N_CORES)
        cores.append(dict(b=b_idx[sel], t=t_idx[sel], lm=lm_f[sel],
                          y=y_f[sel], w=w_f[sel]))
    return cores


def _pack_T(mat, F, pad):
    """[NJ, V] f32 -> [128, NCHUNK*F] fp8 (vocab-on-partitions layout)."""
    NJ = mat.shape[0]
    out = np.full((VP, F), pad, np.float32)
    out[:V, :NJ] = mat.T
    out = out.reshape(NCHUNK, P, F).transpose(1, 0, 2).reshape(P, NCHUNK * F)
    return out.astype(FP8_NP)


def prepare(inputs: dict, variant: str = "v4"):
    logits = np.asarray(inputs["logits"], dtype=np.float32)
    soft = np.asarray(inputs["soft_labels"], dtype=np.float32)
    ys = np.asarray(inputs["ys"])
    aligns = np.asarray(inputs["aligns"])
    xlens = np.asarray(inputs["xlens"])

    cores = _host_prep(ys, aligns, xlens)
    NJ = [len(c["b"]) for c in cores]
    F = -(-max(NJ) // 8) * 8

    key = (F, variant)
    nc = _PROGRAM_CACHE.get(key)
    if nc is None:
        nc = _build_program(F, variant=variant)
        _PROGRAM_CACHE[key] = nc

    in_maps = []
    for c in cores:
        lg = logits[c["b"], c["t"]]                       # [NJ, V]
        sg = soft[c["b"], c["lm"]]                        # [NJ, V]
        slab = W_SOFT * sg + (1.0 - W_SOFT) * A_R
        slab[np.arange(len(c["y"])), c["y"]] += C_Y
        slab *= SCALE
        in_maps.append({
            "r8": _pack_T(np.clip(lg, -240.0, 240.0), F, PAD_LOGIT),
            "s8": _pack_T(slab, F, 0.0),
        })
    return nc, in_maps, cores, NJ, F


def combine(results, cores, NJ, F, variant: str = "v4") -> np.float32:
    total = 0.0
    for c, cinfo in enumerate(cores):
        st = np.asarray(results[c]["stats"], dtype=np.float64).reshape(2, F)
        nj = NJ[c]
        dot = st[0, :nj] / SCALE
        lse = np.log(st[1, :nj])
        contrib = dot - lse
        total += float((cinfo["w"] * contrib).sum())
    return np.float32(-total)


def run(inputs: dict, variant: str = "v4", trace: bool = False, trace_cores=None):
    from concourse.bass_utils import run_bass_kernel_spmd

    nc, in_maps, cores, NJ, F = prepare(inputs, variant)
    res = run_bass_kernel_spmd(
        nc,
        in_maps,
        list(range(N_CORES)),
        trace=trace,
        trace_cores=trace_cores,
    )
    loss = combine(res.results, cores, NJ, F, variant)
    return loss, res


def kernel(**inputs) -> np.ndarray:
    loss, _ = run(inputs)
    return np.asarray(loss, dtype=np.float32)
